# revision 12
# baseline (speedup 1.0000x reference)
"""BitLinear (RMSNorm + int8 absmax activation quant + ternary absmean weight
quant + linear + rescale) on 8 Trainium2 NeuronCores.

Sharding: 2 row-groups x 4 col-groups. Each core gets half the rows of x and a
quarter of the weight rows (out_features), computes its [R/2, O/4] output block;
the host assembles the 8 blocks. The global weight scale (mean|W| over the full
weight) is computed on-device with an AllReduce across the 8 cores.

The weight slice handed to each core is row-permuted so that the core's
disjointly-owned eighth of the full weight comes first: the |w| partial sums
are taken over o-tiles 0..7 while those same tiles are later re-used for
quantization in chunk-consumption order, and the host un-permutes the output
columns when assembling.

The matmul runs in bf16 which is exact here: quantized activations are integers
in [-127, 127] and quantized weights are in {-1, 0, 1}, both exactly
representable in bf16, and fp32 PSUM accumulation of integer products of this
magnitude is exact.

Matmuls are issued chunk-outer (for each PSUM chunk, sweep all 16 k-tiles):
consecutive matmuls then carry *different* stationary operands, so each
LDWEIGHTS prefetches into the PE background weight buffer while the previous
matmul streams (~215 ns/matmul at N=512, vs ~263 ns when the same stationary
tile is redundantly reloaded between chunks).
"""

import sys

sys.path.insert(0, "/opt/trn_rl_repo")

import numpy as np

B, S, D_IN, D_OUT = 4, 2048, 2048, 8192
N_CORES = 8
N_R, N_O = 2, 4
R = B * S // N_R      # rows of x per core
O = D_OUT // N_O      # out cols per core
H = O // 2            # owned half of the o-slice
EPS = 1e-6
MAGIC = 12582912.0    # 1.5 * 2**23: fp32 add/sub round-to-nearest-even trick


def build_nc(rows, d_in, o_cols, n_r, n_o):
    """Build the SPMD bass program for one core."""
    import concourse.tile as tile
    from concourse import bacc, mybir

    f32 = mybir.dt.float32
    bf16 = mybir.dt.bfloat16
    n_cores = n_r * n_o
    P = 128
    n_rt = rows // P            # row tiles
    n_kt = d_in // P            # contraction tiles
    n_ot = o_cols // P          # weight row tiles (out features per core)
    n_own = n_ot // 2           # tiles covered by this core's |w| partial sum
    nch = 512                   # psum chunk (free dim per matmul)
    n_ch = o_cols // nch        # chunks per row tile
    otpc = nch // P             # o-tiles per chunk
    inv_total = 1.0 / ((n_o * o_cols) * d_in)

    nc = bacc.Bacc("TRN2", target_bir_lowering=False, debug=False,
                   num_devices=n_cores)

    x_d = nc.dram_tensor("x", [rows, d_in], f32, kind="ExternalInput").ap()
    w_d = nc.dram_tensor("w", [o_cols, d_in], f32, kind="ExternalInput").ap()
    g_d = nc.dram_tensor("gamma", [d_in], f32, kind="ExternalInput").ap()
    # bf16 output: the host casts back to f32. Output values are integer
    # dot products times a per-row scale; the bf16 rounding adds ~1e-3
    # relative error, well inside the tolerance, and halves output traffic.
    o_d = nc.dram_tensor("out", [rows, o_cols], bf16,
                         kind="ExternalOutput").ap()
    cc_in = nc.dram_tensor("cc_in", [P], f32)
    cc_out = nc.dram_tensor("cc_out", [P], f32, addr_space="Shared")

    with tile.TileContext(nc) as tc:
        with (
            tc.tile_pool(name="gamp", bufs=1) as gamp,
            tc.tile_pool(name="cstp", bufs=1) as cstp,
            tc.tile_pool(name="wsp", bufs=8) as wsp,     # w f32 stream pool
            tc.tile_pool(name="wbp", bufs=3) as wbp,     # w bf16 quant pool
            tc.tile_pool(name="wqtp", bufs=1) as wqtp,   # wqT resident
            tc.tile_pool(name="xp", bufs=2) as xp,
            tc.tile_pool(name="gp", bufs=2) as gp,
            tc.tile_pool(name="xqp", bufs=2) as xqp,
            tc.tile_pool(name="xqtp", bufs=3) as xqtp,
            tc.tile_pool(name="op", bufs=2) as op,
            tc.tile_pool(name="stp", bufs=3) as stp,
            tc.tile_pool(name="psp", bufs=2, space="PSUM") as psp,
        ):
            # ---- constants ----
            mg = cstp.tile([P, 1], f32)
            nc.vector.memset(mg[:], MAGIC)

            # ---- weight phase 1: |w| partial sums over owned tiles 0..7 ----
            # (host puts the owned half of the o-slice first). The staged
            # tiles stay resident and are consumed directly by quantization
            # (they are also the first tiles the matmul needs).
            asum = cstp.tile([P, n_own], f32)
            wq_srcs = []
            for j in range(n_own):
                wt = wsp.tile([P, d_in], f32, tag="wt", name=f"wa_{j}")
                nc.sync.dma_start(wt[:], w_d[j * P:(j + 1) * P, :])
                nc.vector.tensor_reduce(asum[:, j:j + 1], wt[:],
                                        axis=mybir.AxisListType.X,
                                        op=mybir.AluOpType.add,
                                        apply_absolute_value=True)
                wq_srcs.append(wt)
            apart = cstp.tile([P, 1], f32)
            nc.vector.tensor_reduce(apart[:], asum[:],
                                    axis=mybir.AxisListType.X,
                                    op=mybir.AluOpType.add)
            nc.gpsimd.dma_start(cc_in.ap().unsqueeze(1), apart[:])
            nc.gpsimd.collective_compute(
                "AllReduce", mybir.AluOpType.add,
                replica_groups=[list(range(n_cores))],
                ins=[cc_in.ap()], outs=[cc_out.ap()],
            )

            sums = cstp.tile([P, P], f32)
            nc.gpsimd.dma_start(
                sums[:], cc_out.ap().unsqueeze(0).partition_broadcast(P))
            ws_sum = cstp.tile([P, 1], f32)
            nc.vector.tensor_reduce(ws_sum[:], sums[:],
                                    axis=mybir.AxisListType.X,
                                    op=mybir.AluOpType.add)
            w_scale = cstp.tile([P, 1], f32)
            nc.vector.tensor_scalar(w_scale[:], ws_sum[:], inv_total, 1e-5,
                                    op0=mybir.AluOpType.mult,
                                    op1=mybir.AluOpType.max)
            rws = cstp.tile([P, 1], f32)
            nc.vector.reciprocal(rws[:], w_scale[:])
            ws127 = cstp.tile([P, 1], f32)
            nc.vector.tensor_scalar(ws127[:], w_scale[:], 1.0 / 127.0,
                                    None, op0=mybir.AluOpType.mult)

            # Tiles 8..15 stream in behind the staged ones. Their buffers are
            # released only once quantization (gated on rws, hence on the
            # cc_out read) consumes tiles 0..7, so these must sit after the
            # cc_out read on the gpsimd FIFO to avoid a cyclic wait.
            for j in range(n_own, n_ot):
                wt = wsp.tile([P, d_in], f32, tag="wt", name=f"wq_{j}")
                nc.gpsimd.dma_start(wt[:], w_d[j * P:(j + 1) * P, :])
                wq_srcs.append(wt)

            # ---- weight phase 2: quantize + transpose ----
            # wqT[d_in%128, d_tile, o_tile, o%128] = wq[o, d]
            # p1: t = w/ws + MAGIC     (fp32 magic round)
            # p2: u = min(t-(MAGIC-128), 129) -> bf16  (= 128 + min(rn, 1))
            # p3: wq = max(u, 127) - 128               (= clip(rn, -1, 1))
            # u in [125, 129]: exact in bf16 (step 1 up to 256), so the
            # arithmetic stays exact. (Bias 256 would NOT work: 257 is not
            # representable in bf16.)
            wqT = wqtp.tile([P, n_kt, n_ot, P], bf16)
            for j in range(n_ot):
                wt = wq_srcs[j]
                if j % 2 == 0:
                    nc.scalar.activation(wt[:], wt[:],
                                         mybir.ActivationFunctionType.Identity,
                                         bias=mg[:], scale=rws[:])
                else:
                    nc.vector.tensor_scalar(wt[:], wt[:], rws[:], MAGIC,
                                            op0=mybir.AluOpType.mult,
                                            op1=mybir.AluOpType.add)
                wq = wbp.tile([P, d_in], bf16)
                nc.any.tensor_scalar(wq[:], wt[:], MAGIC - 128.0, 129.0,
                                     op0=mybir.AluOpType.subtract,
                                     op1=mybir.AluOpType.min)
                nc.any.tensor_scalar(wq[:], wq[:], 127.0, 128.0,
                                     op0=mybir.AluOpType.max,
                                     op1=mybir.AluOpType.subtract)
                nc.sync.dma_start_transpose(wqT[:, :, j, :], wq[:])

            # ---- constants / gamma for x phase ----
            gam = gamp.tile([P, d_in], f32)
            nc.sync.dma_start(gam[:], g_d.unsqueeze(0).partition_broadcast(P))

            # ---- x phase: rmsnorm + quantize + matmul per row tile ----
            for i in range(n_rt):
                xt = xp.tile([P, d_in], f32)
                nc.sync.dma_start(xt[:], x_d[i * P:(i + 1) * P, :])
                gt = gp.tile([P, d_in], f32)
                ss = stp.tile([P, 1], f32, tag="ss")
                # sum of x^2 along the row (gt is a dump buffer here)
                nc.scalar.activation(gt[:], xt[:],
                                     mybir.ActivationFunctionType.Square,
                                     accum_out=ss[:])
                # gt = x * gamma;  mx = max|gt| along the row
                mx = stp.tile([P, 1], f32, tag="mx")
                nc.vector.tensor_tensor(out=gt[:], in0=xt[:], in1=gam[:],
                                        op=mybir.AluOpType.mult)
                nc.vector.tensor_reduce(mx[:], gt[:], axis=mybir.AxisListType.X,
                                        op=mybir.AluOpType.max,
                                        apply_absolute_value=True)
                # x_scale = max(mx / rms, 1e-5); sq = 127/(rms*x_scale)
                t1 = stp.tile([P, 1], f32, tag="t1")
                nc.vector.tensor_scalar(t1[:], ss[:], 1.0 / d_in, EPS,
                                        op0=mybir.AluOpType.mult,
                                        op1=mybir.AluOpType.add)
                rms = stp.tile([P, 1], f32, tag="rms")
                nc.scalar.activation(rms[:], t1[:],
                                     mybir.ActivationFunctionType.Sqrt)
                r1 = stp.tile([P, 1], f32, tag="r1")
                nc.vector.reciprocal(r1[:], rms[:])
                xsc = stp.tile([P, 1], f32, tag="xsc")
                nc.vector.tensor_scalar(xsc[:], mx[:], r1[:], 1e-5,
                                        op0=mybir.AluOpType.mult,
                                        op1=mybir.AluOpType.max)
                d0 = stp.tile([P, 1], f32, tag="d0")
                nc.vector.tensor_tensor(out=d0[:], in0=rms[:], in1=xsc[:],
                                        op=mybir.AluOpType.mult)
                d1 = stp.tile([P, 1], f32, tag="d1")
                nc.vector.tensor_scalar(d1[:], d0[:], 1.0 / 127.0, None,
                                        op0=mybir.AluOpType.mult)
                sq = stp.tile([P, 1], f32, tag="sq")
                nc.vector.reciprocal(sq[:], d1[:])
                osc = stp.tile([P, 1], f32, tag="osc")
                nc.vector.tensor_scalar(osc[:], xsc[:], ws127[:], None,
                                        op0=mybir.AluOpType.mult)
                # xq = round(gt * sq) via magic add/sub, to bf16
                nc.scalar.activation(gt[:], gt[:],
                                     mybir.ActivationFunctionType.Identity,
                                     bias=mg[:], scale=sq[:])
                xq = xqp.tile([P, d_in], bf16)
                nc.vector.tensor_scalar(xq[:], gt[:], MAGIC, None,
                                        op0=mybir.AluOpType.subtract)
                xqT = xqtp.tile([P, n_kt, P], bf16)
                nc.sync.dma_start_transpose(xqT[:], xq[:])
                # matmul: out[r, o] = sum_d xq[r, d] * wq[o, d]
                # chunk-outer so consecutive matmuls change the stationary
                # operand (LDWEIGHTS prefetches into the background buffer)
                for c in range(n_ch):
                    ps = psp.tile([P, nch], f32, tag=f"ps{c}",
                                  name=f"ps{c}_{i}")
                    for k in range(n_kt):
                        nc.tensor.matmul(
                            ps[:], xqT[:, k, :],
                            wqT[:, k, c * otpc:(c + 1) * otpc, :],
                            start=(k == 0), stop=(k == n_kt - 1))
                    ot = op.tile([P, nch], bf16, tag="oc", name=f"oc_{i}_{c}")
                    nc.scalar.activation(ot[:], ps[:],
                                         mybir.ActivationFunctionType.Copy,
                                         scale=osc[:])
                    nc.sync.dma_start(
                        o_d[i * P:(i + 1) * P, c * nch:(c + 1) * nch], ot[:])

    nc.compile()
    return nc


_cache = {}


def _get_nc():
    if "nc" not in _cache:
        _cache["nc"] = build_nc(R, D_IN, O, N_R, N_O)
    return _cache["nc"]


def make_in_maps(x, weight, gamma):
    """Shard the full inputs into per-core input maps.

    Each core's weight slice is row-permuted so its disjointly-owned half
    (rows [ri*H, (ri+1)*H) of the o-slice) comes first.
    """
    X = np.ascontiguousarray(np.asarray(x, np.float32).reshape(B * S, D_IN))
    W = np.ascontiguousarray(np.asarray(weight, np.float32))
    G = np.ascontiguousarray(np.asarray(gamma, np.float32))

    in_maps = []
    for c in range(N_CORES):
        ri, oj = divmod(c, N_O)
        blk = W[oj * O:(oj + 1) * O]
        own = blk[ri * H:(ri + 1) * H]
        other = blk[(1 - ri) * H:(2 - ri) * H]
        in_maps.append({
            "x": X[ri * R:(ri + 1) * R],
            "w": np.ascontiguousarray(np.concatenate([own, other], axis=0)),
            "gamma": G,
        })
    return in_maps


def assemble_output(results):
    """Gather per-core output blocks, undoing the column permutation."""
    out = np.empty((B * S, D_OUT), np.float32)
    for c in range(N_CORES):
        ri, oj = divmod(c, N_O)
        res = np.asarray(results[c]["out"]).astype(np.float32)
        rs = slice(ri * R, (ri + 1) * R)
        out[rs, oj * O + ri * H:oj * O + (ri + 1) * H] = res[:, :H]
        out[rs, oj * O + (1 - ri) * H:oj * O + (2 - ri) * H] = res[:, H:]
    return out.reshape(B, S, D_OUT)


def kernel(x, weight, gamma):
    from concourse.bass_utils import run_bass_kernel_spmd

    nc = _get_nc()
    in_maps = make_in_maps(x, weight, gamma)
    res = run_bass_kernel_spmd(nc, in_maps, core_ids=list(range(N_CORES)))
    return assemble_output(res.results)
